# revision 37
# baseline (speedup 1.0000x reference)
"""Bass/Trainium2 kernel for a 2-layer bidirectional QRNN (fo-pooling).

Reference computation (per layer, per direction):
    ZFO = X @ W + b            # [S, B, 3H]
    Z, F, O = split(ZFO); Z = tanh(Z); F = sigmoid(F); O = sigmoid(O)
    c_t = F_t * c_{t-1} + (1 - F_t) * Z_t        (bw direction: reversed time)
    Y_dir = O * C
    Y = concat(Y_fw, Y_bw)     # [S, B, 2H]
Two stacked layers; output is [B, S, 2H].

Sharding: data-parallel over batch. B=16 rows -> 2 rows per NeuronCore x 8.
Each core runs both layers for its 2 rows; no collectives.

Device layout: everything is feature-major ([feat, seq] per batch row) so the
matmul (which contracts over the partition axis) needs no on-chip transposes:
layer-0 input is host-pre-transposed X^T, layer-0 output Y1 is produced
feature-major (exactly what layer 1 consumes via a DRAM round-trip), and the
final output is un-transposed on the host.

The time recurrence uses the DVE `tensor_tensor_scan` instruction
(state = f*state + g along the free axis); the bw direction runs the scan
through reversed access patterns with s-tiles processed in descending order,
chaining the carry via a [128,1] column copy.

mm_dtype="fp32r" (default) computes the gate projections in fp32r (TF32-like
10-bit-mantissa rounding, 4x the fp32 PE rate; measured end-to-end relative
error ~3e-4). fp32r operand tiles must be produced by a compute-engine cast:
a fp32r DMA faults the exec unit and a plain bitcast fails BIR verification.
mm_dtype="fp32" is the exact-precision fallback (~3x slower).
"""

import numpy as np

import concourse.bacc as bacc
import concourse.mybir as mybir
from concourse import bass_utils
from concourse.tile import TileContext

# problem dims (hardcoded per spec)
B, S, D, H = 16, 2048, 512, 512
N_CORES = 8
BC = B // N_CORES  # batch rows per core
P = 128  # SBUF partitions
S_TILE = 512

F32 = mybir.dt.float32
ACT = mybir.ActivationFunctionType
ALU = mybir.AluOpType


def build_nc(bc=BC, s=S, d=D, h=H, s_tile=S_TILE, mm_dtype="fp32r"):
    """Build the SPMD Bass program (same program on every core)."""
    nc = bacc.Bacc("TRN2", target_bir_lowering=False)

    xt = nc.dram_tensor("xt", [bc, d, s], F32, kind="ExternalInput")
    w0f = nc.dram_tensor("w0f", [d, 3 * h], F32, kind="ExternalInput")
    w0b = nc.dram_tensor("w0b", [d, 3 * h], F32, kind="ExternalInput")
    b0f = nc.dram_tensor("b0f", [3 * h], F32, kind="ExternalInput")
    b0b = nc.dram_tensor("b0b", [3 * h], F32, kind="ExternalInput")
    w1f = nc.dram_tensor("w1f", [2 * h, 3 * h], F32, kind="ExternalInput")
    w1b = nc.dram_tensor("w1b", [2 * h, 3 * h], F32, kind="ExternalInput")
    b1f = nc.dram_tensor("b1f", [3 * h], F32, kind="ExternalInput")
    b1b = nc.dram_tensor("b1b", [3 * h], F32, kind="ExternalInput")
    y1 = nc.dram_tensor("y1", [bc, 2 * h, s], F32)  # layer-0 out / layer-1 in
    out_t = nc.dram_tensor("out_t", [bc, 2 * h, s], F32, kind="ExternalOutput")

    ns = s // s_tile
    hc = h // P
    mmdt = mybir.dt.float32r if mm_dtype == "fp32r" else F32

    # DMA queue split: input streams and output writes ride the sync HWDGE
    # queue; weights and biases ride the scalar HWDGE queue (the only two HW
    # DGE queues). A dma_start costs ~600ns on the ISSUING engine, and every
    # engine executes its stream in order — so bulk weight loads are broken
    # into small thunks (one DMA issue or one cast each) and drip-fed through
    # the preceding pass's iterations, where they fit into engine slack.
    STAGE_BUFS = 4

    def weight_load_items(pool, stage_pool, wd, k_chunks, prefix):
        """Create the [P, 3h] weight tiles for one (layer, direction) and
        return (tiles, items): items are thunks (DMA issues into shared
        staging slots, interleaved with ScalarE fp32r casts; BIR verification
        requires the matmul operand's producer to be a rounding compute op).
        Emitting an item never blocks — staging-slot recycling only
        back-pressures the scalar DMA queue at runtime."""
        tiles = [
            pool.tile([P, 3 * h], mmdt, tag=f"{prefix}_wk{k}", name=f"{prefix}_wk{k}")
            for k in range(k_chunks)
        ]
        if mmdt is F32:
            items = [
                (lambda k=k: nc.scalar.dma_start(tiles[k][:], wd[k * P : (k + 1) * P, :]))
                for k in range(k_chunks)
            ]
            return tiles, items
        stgs = {}

        def dma_item(k):
            stg = stage_pool.tile([P, 3 * h], F32, tag="wstg", bufs=STAGE_BUFS,
                                  name=f"{prefix}_stg{k}")
            stgs[k] = stg
            nc.scalar.dma_start(stg[:], wd[k * P : (k + 1) * P, :])

        def cast_item(k):
            nc.scalar.copy(tiles[k][:], stgs.pop(k)[:])

        dmas = [(lambda k=k: dma_item(k)) for k in range(k_chunks)]
        casts = [(lambda k=k: cast_item(k)) for k in range(k_chunks)]
        # interleave with a STAGE_BUFS lead so a cast's DMA is long issued
        items = dmas[:STAGE_BUFS]
        for i in range(k_chunks):
            items.append(casts[i])
            if STAGE_BUFS + i < k_chunks:
                items.append(dmas[STAGE_BUFS + i])
        return tiles, items

    def load_biases(pool, bd, prefix):
        """One DMA loads the whole [3h] bias vector as a [P, 3*hc] column
        table; returns per-(gate, h-chunk) [P, 1] views."""
        btab = pool.tile([P, 3 * hc], F32, tag=f"{prefix}_btab", name=f"{prefix}_btab")
        nc.scalar.dma_start(btab[:], bd[:].rearrange("(j p) -> p j", p=P))
        return {
            (g, hh): btab[:, g * hc + hh : g * hc + hh + 1]
            for g in range(3)
            for hh in range(hc)
        }

    def direction_pass(pools, layer, fw, src, wk, btile, dst, drip=()):
        """One (layer, direction) pass over all batch rows.

        src: DRAM input [bc, Din, s] (xt for layer 0, y1 for layer 1).
        dst: DRAM output [bc, 2h, s]; writes rows [dir_off, dir_off + h).
        drip: deferred small thunks (next passes' weight casts), emitted one
              per iteration so they slot into engine-stream slack.
        """
        spool, cpool, ypool, ppool = pools
        drip_iter = iter(drip)
        n_iters = bc * ns
        per_drip = -(-len(drip) // max(n_iters - 1, 1)) if drip else 0
        k_chunks = (d if layer == 0 else 2 * h) // P
        dir_off = 0 if fw else h
        s_order = list(range(ns)) if fw else list(range(ns - 1, -1, -1))
        for b in range(bc):
            carry = [cpool.tile([P, 1], F32, tag=f"c{hh}", name=f"carry{hh}") for hh in range(hc)]
            for si, s_idx in enumerate(s_order):
                s0 = s_idx * s_tile
                ins = []
                for k in range(k_chunks):
                    if mmdt is F32:
                        t = ypool.tile([P, s_tile], F32, tag=f"inr{k}", name=f"in{k}")
                        nc.sync.dma_start(
                            t[:], src[b, k * P : (k + 1) * P, s0 : s0 + s_tile]
                        )
                    else:
                        stg = ypool.tile([P, s_tile], F32, tag="instg", bufs=4, name="instg")
                        nc.sync.dma_start(
                            stg[:], src[b, k * P : (k + 1) * P, s0 : s0 + s_tile]
                        )
                        t = ypool.tile([P, s_tile], mmdt, tag=f"inr{k}", name=f"inr{k}")
                        # layer 0 is DVE-paced: push one cast per iteration
                        # to ScalarE, which has slack there
                        cast_eng = nc.scalar if (layer == 0 and k == 0) else nc.vector
                        if cast_eng is nc.scalar:
                            cast_eng.copy(t[:], stg[:])
                        else:
                            cast_eng.tensor_copy(t[:], stg[:])
                    ins.append(t[:])
                for hh in range(hc):
                    ps = [
                        ppool.tile([P, s_tile], F32, tag=f"ps{g}", name=f"ps{g}",
                                   bufs=(3 if g < 2 else 2))
                        for g in range(3)
                    ]
                    for g in range(3):
                        cols = slice(g * h + hh * P, g * h + (hh + 1) * P)
                        for k in range(k_chunks):
                            nc.tensor.matmul(
                                ps[g][:],
                                wk[k][:, cols],
                                ins[k],
                                start=(k == 0),
                                stop=(k == k_chunks - 1),
                            )
                    z = spool.tile([P, s_tile], F32, tag="z", name="z")
                    f_ = spool.tile([P, s_tile], F32, tag="f", name="f")
                    o = spool.tile([P, s_tile], F32, tag="o", name="o")
                    fn = spool.tile([P, s_tile], F32, tag="fn", name="fn")
                    g_ = spool.tile([P, s_tile], F32, tag="g", name="g")
                    c = spool.tile([P, s_tile], F32, tag="c", name="c")
                    y = spool.tile([P, s_tile], F32, tag="y", name="y")
                    nc.scalar.activation(z[:], ps[0][:], ACT.Tanh, bias=btile[0, hh][:])
                    nc.scalar.activation(f_[:], ps[1][:], ACT.Sigmoid, bias=btile[1, hh][:])
                    nc.scalar.activation(o[:], ps[2][:], ACT.Sigmoid, bias=btile[2, hh][:])
                    # g = (1 - f) * z   (1-f on the otherwise idle GPSIMD)
                    nc.gpsimd.tensor_scalar(fn[:], f_[:], -1.0, 1.0, ALU.mult, ALU.add)
                    g_eng = nc.gpsimd if (layer == 0 and hh == 3) else nc.vector
                    g_eng.tensor_mul(g_[:], fn[:], z[:])
                    # c_t = f_t * c_prev + g_t (bw: time runs backwards)
                    if fw:
                        sc = (c[:], f_[:], g_[:])
                        carry_col = slice(s_tile - 1, s_tile)
                    else:
                        sc = (c[:, ::-1], f_[:, ::-1], g_[:, ::-1])
                        carry_col = slice(0, 1)
                    init = 0.0 if si == 0 else carry[hh][:]
                    nc.vector.tensor_tensor_scan(
                        sc[0], sc[1], sc[2], init, ALU.mult, ALU.add
                    )
                    if si < ns - 1:
                        nc.gpsimd.tensor_copy(carry[hh][:], c[:, carry_col])
                    nc.gpsimd.tensor_mul(y[:], o[:], c[:])
                    row0 = dir_off + hh * P
                    nc.sync.dma_start(dst[b, row0 : row0 + P, s0 : s0 + s_tile], y[:])
                if b > 0 or si > 0:
                    for _ in range(per_drip):
                        thunk = next(drip_iter, None)
                        if thunk is not None:
                            thunk()
        for thunk in drip_iter:
            thunk()

    with TileContext(nc) as tc:
        # One shared set of working pools across all four passes -- pool
        # boundaries create SBUF-address-reuse barriers (the next pass's
        # first tiles wait for the old pool's last readers), while shared
        # tags hand off slot-by-slot and keep the PE fed.
        with (
            tc.tile_pool(name="shared", bufs=1) as shpool,
            tc.tile_pool(name="scr", bufs=3) as spool,
            tc.tile_pool(name="carry", bufs=1) as cpool,
            tc.tile_pool(name="instream", bufs=2) as ypool,
            tc.tile_pool(name="ps", bufs=2, space="PSUM") as ppool,
            tc.tile_pool(name="w1f_pool", bufs=1) as w1f_pool,
        ):
            pools = (spool, cpool, ypool, ppool)

            # -------- layer 0 (streams xt, writes y1) --------
            with tc.tile_pool(name="w0_pool", bufs=1) as w0_pool:
                # w0f loads+casts run immediately (they gate the kernel
                # start); everything else is issued up front on the scalar
                # queue but cast via drip.
                wk0f, it0f = weight_load_items(w0_pool, shpool, w0f, d // P, "w0f")
                for it in it0f:
                    it()
                bt0f = load_biases(w0_pool, b0f, "w0f")
                bt0b = load_biases(w0_pool, b0b, "w0b")
                bt1f = load_biases(w1f_pool, b1f, "w1f")
                wk0b, it0b = weight_load_items(w0_pool, shpool, w0b, d // P, "w0b")
                wk1f, it1f = weight_load_items(w1f_pool, shpool, w1f, 2 * h // P, "w1f")
                direction_pass(pools, 0, True, xt, wk0f, bt0f, y1, drip=it0b + it1f[:4])
                direction_pass(pools, 0, False, xt, wk0b, bt0b, y1, drip=it1f[4:])

            # -------- layer 1 (streams y1, writes out_t) --------
            with tc.tile_pool(name="w1b_pool", bufs=1) as w1b_pool:
                bt1b = load_biases(w1b_pool, b1b, "w1b")
                wk1b, it1b = weight_load_items(w1b_pool, shpool, w1b, 2 * h // P, "w1b")
                direction_pass(pools, 1, True, y1, wk1f, bt1f, out_t, drip=it1b)
                direction_pass(pools, 1, False, y1, wk1b, bt1b, out_t)

    nc.finalize()
    return nc


_NC_CACHE = {}


def _get_nc(mm_dtype):
    if mm_dtype not in _NC_CACHE:
        _NC_CACHE[mm_dtype] = build_nc(mm_dtype=mm_dtype)
    return _NC_CACHE[mm_dtype]


def kernel(X, seqlens, W_fw0, b_fw0, W_bw0, b_bw0, W_fw1, b_fw1, W_bw1, b_bw1,
           mm_dtype="fp32r", trace=False):
    """Full-input entry point: shards over 8 cores, returns [B, S, 2H] f32."""
    del seqlens  # unused by the reference computation
    X = np.ascontiguousarray(np.asarray(X, dtype=np.float32))
    weights = {
        "w0f": W_fw0, "b0f": b_fw0, "w0b": W_bw0, "b0b": b_bw0,
        "w1f": W_fw1, "b1f": b_fw1, "w1b": W_bw1, "b1b": b_bw1,
    }
    weights = {k: np.ascontiguousarray(np.asarray(v, dtype=np.float32))
               for k, v in weights.items()}

    nc = _get_nc(mm_dtype)
    in_maps = []
    for i in range(N_CORES):
        rows = X[i * BC : (i + 1) * BC]  # [BC, S, D]
        xt_i = np.ascontiguousarray(rows.transpose(0, 2, 1))  # [BC, D, S]
        in_maps.append({"xt": xt_i, **weights})

    res = bass_utils.run_bass_kernel_spmd(
        nc, in_maps, core_ids=list(range(N_CORES)), trace=trace
    )
    out = np.empty((B, S, 2 * H), dtype=np.float32)
    for i in range(N_CORES):
        out_t = res.results[i]["out_t"]  # [BC, 2H, S]
        out[i * BC : (i + 1) * BC] = out_t.transpose(0, 2, 1)
    kernel.last_results = res
    return out


# revision 38
# speedup vs baseline: 1.0414x; 1.0414x over previous
"""Bass/Trainium2 kernel for a 2-layer bidirectional QRNN (fo-pooling).

Reference computation (per layer, per direction):
    ZFO = X @ W + b            # [S, B, 3H]
    Z, F, O = split(ZFO); Z = tanh(Z); F = sigmoid(F); O = sigmoid(O)
    c_t = F_t * c_{t-1} + (1 - F_t) * Z_t        (bw direction: reversed time)
    Y_dir = O * C
    Y = concat(Y_fw, Y_bw)     # [S, B, 2H]
Two stacked layers; output is [B, S, 2H].

Sharding: data-parallel over batch. B=16 rows -> 2 rows per NeuronCore x 8.
Each core runs both layers for its 2 rows; no collectives.

Device layout: everything is feature-major ([feat, seq] per batch row) so the
matmul (which contracts over the partition axis) needs no on-chip transposes:
layer-0 input is host-pre-transposed X^T, layer-0 output Y1 is produced
feature-major (exactly what layer 1 consumes via a DRAM round-trip), and the
final output is un-transposed on the host.

The time recurrence uses the DVE `tensor_tensor_scan` instruction
(state = f*state + g along the free axis); the bw direction runs the scan
through reversed access patterns with s-tiles processed in descending order,
chaining the carry via a [128,1] column copy.

mm_dtype="fp32r" (default) computes the gate projections in fp32r (TF32-like
10-bit-mantissa rounding, 4x the fp32 PE rate; measured end-to-end relative
error ~3e-4). fp32r operand tiles must be produced by a compute-engine cast:
a fp32r DMA faults the exec unit and a plain bitcast fails BIR verification.
mm_dtype="fp32" is the exact-precision fallback (~3x slower).
"""

import numpy as np

import concourse.bacc as bacc
import concourse.mybir as mybir
from concourse import bass_utils
from concourse.tile import TileContext

# problem dims (hardcoded per spec)
B, S, D, H = 16, 2048, 512, 512
N_CORES = 8
BC = B // N_CORES  # batch rows per core
P = 128  # SBUF partitions
S_TILE = 512

F32 = mybir.dt.float32
ACT = mybir.ActivationFunctionType
ALU = mybir.AluOpType


def build_nc(bc=BC, s=S, d=D, h=H, s_tile=S_TILE, mm_dtype="fp32r"):
    """Build the SPMD Bass program (same program on every core)."""
    nc = bacc.Bacc("TRN2", target_bir_lowering=False)

    xt = nc.dram_tensor("xt", [bc, d, s], F32, kind="ExternalInput")
    w0f = nc.dram_tensor("w0f", [d, 3 * h], F32, kind="ExternalInput")
    w0b = nc.dram_tensor("w0b", [d, 3 * h], F32, kind="ExternalInput")
    b0f = nc.dram_tensor("b0f", [3 * h], F32, kind="ExternalInput")
    b0b = nc.dram_tensor("b0b", [3 * h], F32, kind="ExternalInput")
    w1f = nc.dram_tensor("w1f", [2 * h, 3 * h], F32, kind="ExternalInput")
    w1b = nc.dram_tensor("w1b", [2 * h, 3 * h], F32, kind="ExternalInput")
    b1f = nc.dram_tensor("b1f", [3 * h], F32, kind="ExternalInput")
    b1b = nc.dram_tensor("b1b", [3 * h], F32, kind="ExternalInput")
    y1 = nc.dram_tensor("y1", [bc, 2 * h, s], F32)  # layer-0 out / layer-1 in
    out_t = nc.dram_tensor("out_t", [bc, 2 * h, s], F32, kind="ExternalOutput")

    ns = s // s_tile
    hc = h // P
    mmdt = mybir.dt.float32r if mm_dtype == "fp32r" else F32

    # DMA queue split: input streams and output writes ride the sync HWDGE
    # queue; weights and biases ride the scalar HWDGE queue (the only two HW
    # DGE queues). A dma_start costs ~600ns on the ISSUING engine, and every
    # engine executes its stream in order — so bulk weight loads are broken
    # into small thunks (one DMA issue or one cast each) and drip-fed through
    # the preceding pass's iterations, where they fit into engine slack.
    STAGE_BUFS = 4

    def weight_load_items(pool, stage_pool, wd, k_chunks, prefix):
        """Create the [P, 3h] weight tiles for one (layer, direction) and
        return (tiles, items): items are thunks (DMA issues into shared
        staging slots, interleaved with ScalarE fp32r casts; BIR verification
        requires the matmul operand's producer to be a rounding compute op).
        Emitting an item never blocks — staging-slot recycling only
        back-pressures the scalar DMA queue at runtime."""
        tiles = [
            pool.tile([P, 3 * h], mmdt, tag=f"{prefix}_wk{k}", name=f"{prefix}_wk{k}")
            for k in range(k_chunks)
        ]
        if mmdt is F32:
            items = [
                (lambda k=k: nc.scalar.dma_start(tiles[k][:], wd[k * P : (k + 1) * P, :]))
                for k in range(k_chunks)
            ]
            return tiles, items
        stgs = {}

        def dma_item(k):
            stg = stage_pool.tile([P, 3 * h], F32, tag="wstg", bufs=STAGE_BUFS,
                                  name=f"{prefix}_stg{k}")
            stgs[k] = stg
            nc.scalar.dma_start(stg[:], wd[k * P : (k + 1) * P, :])

        def cast_item(k):
            nc.scalar.copy(tiles[k][:], stgs.pop(k)[:])

        dmas = [(lambda k=k: dma_item(k)) for k in range(k_chunks)]
        casts = [(lambda k=k: cast_item(k)) for k in range(k_chunks)]
        # interleave with a STAGE_BUFS lead so a cast's DMA is long issued
        items = dmas[:STAGE_BUFS]
        for i in range(k_chunks):
            items.append(casts[i])
            if STAGE_BUFS + i < k_chunks:
                items.append(dmas[STAGE_BUFS + i])
        return tiles, items

    def load_biases(pool, bd, prefix):
        """One DMA loads the whole [3h] bias vector as a [P, 3*hc] column
        table; returns per-(gate, h-chunk) [P, 1] views."""
        btab = pool.tile([P, 3 * hc], F32, tag=f"{prefix}_btab", name=f"{prefix}_btab")
        nc.scalar.dma_start(btab[:], bd[:].rearrange("(j p) -> p j", p=P))
        return {
            (g, hh): btab[:, g * hc + hh : g * hc + hh + 1]
            for g in range(3)
            for hh in range(hc)
        }

    def direction_pass(pools, layer, fw, src, wk, btile, dst, drip=()):
        """One (layer, direction) pass over all batch rows.

        src: DRAM input [bc, Din, s] (xt for layer 0, y1 for layer 1).
        dst: DRAM output [bc, 2h, s]; writes rows [dir_off, dir_off + h).
        drip: deferred small thunks (next passes' weight casts), emitted one
              per iteration so they slot into engine-stream slack.
        """
        spool, cpool, ypool, ppool = pools
        drip_iter = iter(drip)
        n_iters = bc * ns
        per_drip = -(-len(drip) // max(n_iters - 1, 1)) if drip else 0
        k_chunks = (d if layer == 0 else 2 * h) // P
        dir_off = 0 if fw else h
        s_order = list(range(ns)) if fw else list(range(ns - 1, -1, -1))
        for b in range(bc):
            carry = [cpool.tile([P, 1], F32, tag=f"c{hh}", name=f"carry{hh}") for hh in range(hc)]
            for si, s_idx in enumerate(s_order):
                s0 = s_idx * s_tile
                ins = []
                for k in range(k_chunks):
                    if mmdt is F32:
                        t = ypool.tile([P, s_tile], F32, tag=f"inr{k}", name=f"in{k}")
                        nc.sync.dma_start(
                            t[:], src[b, k * P : (k + 1) * P, s0 : s0 + s_tile]
                        )
                    else:
                        stg = ypool.tile([P, s_tile], F32, tag="instg", bufs=4, name="instg")
                        nc.sync.dma_start(
                            stg[:], src[b, k * P : (k + 1) * P, s0 : s0 + s_tile]
                        )
                        t = ypool.tile([P, s_tile], mmdt, tag=f"inr{k}", name=f"inr{k}")
                        nc.vector.tensor_copy(t[:], stg[:])
                    ins.append(t[:])
                for hh in range(hc):
                    ps = [
                        ppool.tile([P, s_tile], F32, tag=f"ps{g}", name=f"ps{g}",
                                   bufs=(3 if g < 2 else 2))
                        for g in range(3)
                    ]
                    for g in range(3):
                        cols = slice(g * h + hh * P, g * h + (hh + 1) * P)
                        for k in range(k_chunks):
                            nc.tensor.matmul(
                                ps[g][:],
                                wk[k][:, cols],
                                ins[k],
                                start=(k == 0),
                                stop=(k == k_chunks - 1),
                            )
                    z = spool.tile([P, s_tile], F32, tag="z", name="z")
                    f_ = spool.tile([P, s_tile], F32, tag="f", name="f")
                    o = spool.tile([P, s_tile], F32, tag="o", name="o")
                    fn = spool.tile([P, s_tile], F32, tag="fn", name="fn")
                    g_ = spool.tile([P, s_tile], F32, tag="g", name="g")
                    c = spool.tile([P, s_tile], F32, tag="c", name="c")
                    y = spool.tile([P, s_tile], F32, tag="y", name="y")
                    nc.scalar.activation(z[:], ps[0][:], ACT.Tanh, bias=btile[0, hh][:])
                    nc.scalar.activation(f_[:], ps[1][:], ACT.Sigmoid, bias=btile[1, hh][:])
                    nc.scalar.activation(o[:], ps[2][:], ACT.Sigmoid, bias=btile[2, hh][:])
                    # g = (1 - f) * z   (1-f on the otherwise idle GPSIMD)
                    nc.gpsimd.tensor_scalar(fn[:], f_[:], -1.0, 1.0, ALU.mult, ALU.add)
                    nc.vector.tensor_mul(g_[:], fn[:], z[:])
                    # c_t = f_t * c_prev + g_t (bw: time runs backwards)
                    if fw:
                        sc = (c[:], f_[:], g_[:])
                        carry_col = slice(s_tile - 1, s_tile)
                    else:
                        sc = (c[:, ::-1], f_[:, ::-1], g_[:, ::-1])
                        carry_col = slice(0, 1)
                    init = 0.0 if si == 0 else carry[hh][:]
                    nc.vector.tensor_tensor_scan(
                        sc[0], sc[1], sc[2], init, ALU.mult, ALU.add
                    )
                    if si < ns - 1:
                        nc.gpsimd.tensor_copy(carry[hh][:], c[:, carry_col])
                    nc.gpsimd.tensor_mul(y[:], o[:], c[:])
                    row0 = dir_off + hh * P
                    nc.sync.dma_start(dst[b, row0 : row0 + P, s0 : s0 + s_tile], y[:])
                if b > 0 or si > 0:
                    for _ in range(per_drip):
                        thunk = next(drip_iter, None)
                        if thunk is not None:
                            thunk()
        for thunk in drip_iter:
            thunk()

    with TileContext(nc) as tc:
        # One shared set of working pools across all four passes -- pool
        # boundaries create SBUF-address-reuse barriers (the next pass's
        # first tiles wait for the old pool's last readers), while shared
        # tags hand off slot-by-slot and keep the PE fed.
        with (
            tc.tile_pool(name="shared", bufs=1) as shpool,
            tc.tile_pool(name="scr", bufs=3) as spool,
            tc.tile_pool(name="carry", bufs=1) as cpool,
            tc.tile_pool(name="instream", bufs=2) as ypool,
            tc.tile_pool(name="ps", bufs=2, space="PSUM") as ppool,
            tc.tile_pool(name="w1f_pool", bufs=1) as w1f_pool,
        ):
            pools = (spool, cpool, ypool, ppool)

            # -------- layer 0 (streams xt, writes y1) --------
            with tc.tile_pool(name="w0_pool", bufs=1) as w0_pool:
                # w0f loads+casts run immediately (they gate the kernel
                # start); everything else is issued up front on the scalar
                # queue but cast via drip.
                wk0f, it0f = weight_load_items(w0_pool, shpool, w0f, d // P, "w0f")
                for it in it0f:
                    it()
                bt0f = load_biases(w0_pool, b0f, "w0f")
                bt0b = load_biases(w0_pool, b0b, "w0b")
                bt1f = load_biases(w1f_pool, b1f, "w1f")
                wk0b, it0b = weight_load_items(w0_pool, shpool, w0b, d // P, "w0b")
                wk1f, it1f = weight_load_items(w1f_pool, shpool, w1f, 2 * h // P, "w1f")
                direction_pass(pools, 0, True, xt, wk0f, bt0f, y1, drip=it0b + it1f[:4])
                direction_pass(pools, 0, False, xt, wk0b, bt0b, y1, drip=it1f[4:])

            # -------- layer 1 (streams y1, writes out_t) --------
            with tc.tile_pool(name="w1b_pool", bufs=1) as w1b_pool:
                bt1b = load_biases(w1b_pool, b1b, "w1b")
                wk1b, it1b = weight_load_items(w1b_pool, shpool, w1b, 2 * h // P, "w1b")
                direction_pass(pools, 1, True, y1, wk1f, bt1f, out_t, drip=it1b)
                direction_pass(pools, 1, False, y1, wk1b, bt1b, out_t)

    nc.finalize()
    return nc


_NC_CACHE = {}


def _get_nc(mm_dtype):
    if mm_dtype not in _NC_CACHE:
        _NC_CACHE[mm_dtype] = build_nc(mm_dtype=mm_dtype)
    return _NC_CACHE[mm_dtype]


def kernel(X, seqlens, W_fw0, b_fw0, W_bw0, b_bw0, W_fw1, b_fw1, W_bw1, b_bw1,
           mm_dtype="fp32r", trace=False):
    """Full-input entry point: shards over 8 cores, returns [B, S, 2H] f32."""
    del seqlens  # unused by the reference computation
    X = np.ascontiguousarray(np.asarray(X, dtype=np.float32))
    weights = {
        "w0f": W_fw0, "b0f": b_fw0, "w0b": W_bw0, "b0b": b_bw0,
        "w1f": W_fw1, "b1f": b_fw1, "w1b": W_bw1, "b1b": b_bw1,
    }
    weights = {k: np.ascontiguousarray(np.asarray(v, dtype=np.float32))
               for k, v in weights.items()}

    nc = _get_nc(mm_dtype)
    in_maps = []
    for i in range(N_CORES):
        rows = X[i * BC : (i + 1) * BC]  # [BC, S, D]
        xt_i = np.ascontiguousarray(rows.transpose(0, 2, 1))  # [BC, D, S]
        in_maps.append({"xt": xt_i, **weights})

    res = bass_utils.run_bass_kernel_spmd(
        nc, in_maps, core_ids=list(range(N_CORES)), trace=trace
    )
    out = np.empty((B, S, 2 * H), dtype=np.float32)
    for i in range(N_CORES):
        out_t = res.results[i]["out_t"]  # [BC, 2H, S]
        out[i * BC : (i + 1) * BC] = out_t.transpose(0, 2, 1)
    kernel.last_results = res
    return out
